# revision 22
# baseline (speedup 1.0000x reference)
"""Trainium2 Bass kernel for relative-position multi-head attention.

Shapes (hardcoded): B=2, L=384, D=256, H=8, DH=32.
Sharding: 8 cores; core c handles batch b=c//4, query rows [(c%4)*96, +96).
Pure data-parallel SPMD - no collectives.

Math (per batch b, query q):
  q/k/v projections: x @ W.T + bias
  A_C[h,k] = (q_h+u_h) . k_h[k]
  B_D[h,k] = (q_h+v_h) . (Wr_h @ pos[q,k] + br_h)
           = (Wr_h^T (q_h+v_h)) . pos[q,k]   + const(h,q)   [br term is
             k-independent -> cancels in softmax -> dropped]
  score    = (A_C + B_D)/sqrt(DH) - (1-mask[k])*1e15
  out      = softmax_k(score) @ v

Key restructurings for the hardware:
  * r = pos @ Wr.T (38 GFLOP) is never materialized; instead
    T[q] = Wr^T-blockdiag @ (q+v)  (a [256,8] matrix per query) and
    B_D = posT @ T  (1.2 GFLOP).
  * scores live in PSUM as [k-partitions, (pair,h)-free]; softmax over k
    (partitions) uses exp on ACT + a ones-column appended to v_proj so the
    softmax denominator falls out of the output matmul for free.
  * A_C is folded into the same PSUM accumulation as B_D using a
    block-diagonal (q+u) weight matrix, contracted against k_projT.
  * pos DMA is fully contiguous (3KB per partition line): partition p of a
    pair's slab holds k rows {3p, 3p+1, 3p+2}. All k-indexed tensors
    (key, value, mask) are loaded with the matching k = 3p + j permutation,
    which is invariant for softmax/attention as long as it is consistent.
  * pos is cast f32->bf16 (round-robin Pool/DVE/ACT), then transposed on PE
    as u32-PACKED data (pairs of bf16 d-columns viewed as one f32 element):
    3 [128,128] transposes cover a pair's whole [384,256] slab. B_D matmuls
    contract the packed posT with d-parity-split T matrices.
  * pair loop runs in two phases (pairs 0-63, 64-95) so the scores PSUM
    needs only 3 banks, freeing banks for a 3-deep posT psum ring; B_D of
    pair p is emitted after pair p+1's transposes (software pipelining, no
    in-order PE queue stalls on the PSUM->SBUF copy).
"""

import sys

for _p in ("/opt/trn_rl_repo", "/root/.axon_site/_ro/trn_rl_repo"):
    if _p not in sys.path:
        sys.path.append(_p)

import numpy as np

import concourse.bass as bass
import concourse.mybir as mybir
import concourse.tile as tile
from concourse import bacc
from concourse.masks import make_identity

FP32 = mybir.dt.float32
FP32R = mybir.dt.float32r
BF16 = mybir.dt.bfloat16

B, L, D, H = 2, 384, 256, 8
DH = D // H            # 32
Q = 96                 # queries per core
KT = L // 128          # 3 k-tiles
CB = D // 128          # 2 contraction blocks
NCORES = 8
SCALE = 1.0 / np.sqrt(DH)


def build_kernel_body(tc, outs, ins):
    """Emit the per-core program. outs/ins are dicts of DRAM APs."""
    from contextlib import ExitStack
    ctx = ExitStack()
    pool = lambda **kw: ctx.enter_context(tc.tile_pool(**kw))
    nc = tc.nc
    pos = ins["pos"]          # [Q, L, D] f32
    key = ins["key"]          # [L, D]
    value = ins["value"]      # [L, D]
    query = ins["query"]      # [Q, D]
    mask = ins["mask"]        # [L]
    Wk, Wq, Wv, Wr = ins["Wk"], ins["Wq"], ins["Wv"], ins["Wr"]   # [D, D]
    bk, bq, bv = ins["bk"], ins["bq"], ins["bv"]                  # [D]
    u_in, v_in = ins["u"], ins["v"]                               # [H, DH]
    out = outs["out"]         # [Q, D] f32

    # PSUM budget (8 banks): psum_big 2 + scores 3 + psum_posT 3
    const = pool(name="const", bufs=1)
    setup = pool(name="setup", bufs=2)
    psum_big = pool(name="psum_big", bufs=2, space="PSUM")
    pair_pool = pool(name="pair", bufs=4)
    posT_pool = pool(name="posT", bufs=4)
    cast_pool = pool(name="cast", bufs=8)
    psum_posT = pool(name="psum_posT", bufs=3, space="PSUM")
    scores_pool = pool(name="scores", bufs=3, space="PSUM")

    # ---------------- identities ----------------
    ident_f = const.tile([128, 128], FP32)
    make_identity(nc, ident_f)

    # ---------------- load weights + inputs ----------------
    # setup loads go on the ACT hwdge queue so they are not stuck behind the
    # 1.5MB pos group DMAs on the SP queue
    def load_2tiles(ap, cols, tg):  # [256, cols] dram -> 2 sbuf tiles
        ts = []
        for i in range(2):
            t = setup.tile([128, cols], FP32, tag=f"ld_{tg}{i}",
                           name=f"ld_{tg}{i}")
            nc.scalar.dma_start(out=t, in_=ap[i * 128:(i + 1) * 128, :])
            ts.append(t)
        return ts

    Wk_n = load_2tiles(Wk, D, "wk")
    Wq_n = load_2tiles(Wq, D, "wq")
    Wv_n = load_2tiles(Wv, D, "wv")
    # Wr loaded per-head so matmul lhsT slices start at partition 0
    Wr_h = [const.tile([DH, D], FP32, tag=f"wrh{h}", name=f"wrh{h}")
            for h in range(H)]
    for h in range(H):
        nc.scalar.dma_start(out=Wr_h[h], in_=Wr[h * DH:(h + 1) * DH, :])

    # key/value rows permuted to k = 3p + j so tile j partition p matches the
    # contiguous pos load below.
    key_perm = key.rearrange("(p t) d -> t p d", t=KT)
    val_perm = value.rearrange("(p t) d -> t p d", t=KT)
    key_n = [setup.tile([128, D], FP32, tag=f"ld_key{j}", name=f"keyn{j}") for j in range(KT)]
    val_n = [setup.tile([128, D], FP32, tag=f"vn{j}", name=f"vn{j}") for j in range(KT)]
    for j in range(KT):
        nc.scalar.dma_start(out=key_n[j], in_=key_perm[j])
        nc.scalar.dma_start(out=val_n[j], in_=val_perm[j])
    qry_n = setup.tile([96, D], FP32)
    nc.scalar.dma_start(out=qry_n, in_=query)

    def col_load(ap1d, n, tag):  # [n] dram -> list of [128,1] sbuf columns
        cols = []
        for i in range(0, n, 128):
            c = const.tile([128, 1], FP32, tag=f"col_{tag}{i}", name=f"col_{tag}{i}")
            nc.gpsimd.dma_start(
                out=c, in_=ap1d[i:i + 128].rearrange("(p o) -> p o", o=1))
            cols.append(c)
        return cols

    bk_c = col_load(bk, D, "bk")
    bq_c = col_load(bq, D, "bq")
    u_c = col_load(u_in.rearrange("h d -> (h d)"), D, "u")
    v_c = col_load(v_in.rearrange("h d -> (h d)"), D, "v")
    # mask permuted to k = 3p + j, matching key/value/pos chunk layout
    mask_perm = mask.rearrange("(p t o) -> t p o", t=KT, o=1)
    mask_c = []
    for j in range(KT):
        c = const.tile([128, 1], FP32, tag=f"col_m{j}", name=f"col_m{j}")
        nc.gpsimd.dma_start(out=c, in_=mask_perm[j])
        mask_c.append(c)
    bv_row = const.tile([1, D], FP32)
    nc.gpsimd.dma_start(out=bv_row, in_=bv.rearrange("(o d) -> o d", o=1))

    # mask bias column: (mask-1)*1e15
    mbias = []
    for kt in range(KT):
        mb = const.tile([128, 1], FP32, tag=f"mb{kt}", name=f"mb{kt}")
        nc.vector.tensor_scalar(
            out=mb, in0=mask_c[kt], scalar1=-1.0, scalar2=1e15,
            op0=mybir.AluOpType.add, op1=mybir.AluOpType.mult)
        mbias.append(mb)

    # ---------------- transpose helper (fp32, PE) ----------------
    def transpose_to(dst_tiles, src_tiles, rows, cols, tag):
        """src: list of sbuf tiles [<=128, cols] covering [rows, cols].
        dst_tiles: list of CB sbuf tiles [128, rows] covering [cols, rows]."""
        for cb in range(cols // 128):
            ps = psum_big.tile([128, 512], FP32, tag="big", name="ps_tp")
            nrt = len(src_tiles)
            for i, st in enumerate(src_tiles):
                r = st.shape[0]
                nc.tensor.matmul(
                    ps[:, i * 128:i * 128 + r],
                    st[:, cb * 128:(cb + 1) * 128],
                    ident_f[:r, :r],
                    is_transpose=True,
                    start=(i == 0), stop=(i == nrt - 1))
            nc.vector.tensor_copy(out=dst_tiles[cb], in_=ps[:, :rows])

    keyT = [setup.tile([128, L], FP32, tag=f"keyT{i}", name=f"keyT{i}") for i in range(CB)]
    transpose_to(keyT, key_n, L, D, "k")
    valT = [setup.tile([128, L], FP32, tag=f"valT{i}", name=f"valT{i}") for i in range(CB)]
    transpose_to(valT, val_n, L, D, "v")
    qryT = [setup.tile([128, Q], FP32, tag=f"qryT{i}", name=f"qryT{i}") for i in range(CB)]
    transpose_to(qryT, [qry_n], Q, D, "q")
    WkT = [setup.tile([128, D], FP32, tag=f"WkT{i}", name=f"WkT{i}") for i in range(CB)]
    transpose_to(WkT, Wk_n, D, D, "wk")
    WqT = [setup.tile([128, D], FP32, tag=f"WqT{i}", name=f"WqT{i}") for i in range(CB)]
    transpose_to(WqT, Wq_n, D, D, "wq")
    WvT = [setup.tile([128, D], FP32, tag=f"WvT{i}", name=f"WvT{i}") for i in range(CB)]
    transpose_to(WvT, Wv_n, D, D, "wv")

    # ---------------- projections ----------------
    # k_projT per-head [32, L] bf16 (matmul lhsT base must be 0/32/64)
    kp_h = [const.tile([DH, L], BF16, tag=f"kph{h}", name=f"kph{h}")
            for h in range(H)]
    for dt in range(2):
        ps = psum_big.tile([128, L], FP32, tag="big", name="ps_proj")
        for cb in range(CB):
            nc.tensor.matmul(
                ps, WkT[cb][:, dt * 128:(dt + 1) * 128], keyT[cb],
                start=(cb == 0), stop=(cb == CB - 1))
        for hh in range(4):
            h = dt * 4 + hh
            nc.vector.tensor_scalar_add(
                out=kp_h[h], in0=ps[hh * DH:(hh + 1) * DH, :],
                scalar1=bk_c[dt][hh * DH:(hh + 1) * DH])

    # q_projT [d', q] f32, then qu = +u, qv = +v (per-partition adds)
    quT, qvT = [], []
    for dt in range(2):
        ps = psum_big.tile([128, Q], FP32, tag="big", name="ps_projq")
        for cb in range(CB):
            nc.tensor.matmul(
                ps, WqT[cb][:, dt * 128:(dt + 1) * 128], qryT[cb],
                start=(cb == 0), stop=(cb == CB - 1))
        qp = setup.tile([128, Q], FP32, tag=f"qp{dt}", name=f"qp{dt}")
        nc.vector.tensor_scalar_add(out=qp, in0=ps, scalar1=bq_c[dt])
        qu = const.tile([128, Q], FP32, tag=f"qu{dt}", name=f"qu{dt}")
        nc.vector.tensor_scalar_add(out=qu, in0=qp, scalar1=u_c[dt])
        qv = const.tile([128, Q], FP32, tag=f"qv{dt}", name=f"qv{dt}")
        nc.vector.tensor_scalar_add(out=qv, in0=qp, scalar1=v_c[dt])
        quT.append(qu)
        qvT.append(qv)

    # v_proj natural [k, d'] + ones column per head -> v_aug [128, H*(DH+1)] bf16
    ones_1 = const.tile([1, D], FP32)
    nc.vector.memset(ones_1, 1.0)
    v_aug = []
    for kt in range(KT):
        ps = psum_big.tile([128, D], FP32, tag="big", name="ps_projv")
        for cb in range(CB):
            nc.tensor.matmul(
                ps, valT[cb][:, kt * 128:(kt + 1) * 128], WvT[cb],
                start=(cb == 0), stop=False)
        # + bias bv broadcast over rows (rank-1 matmul with ones lhsT)
        nc.tensor.matmul(ps, ones_1[:, :128], bv_row, start=False, stop=True)
        va = const.tile([128, H, DH + 1], BF16, tag=f"va{kt}", name=f"va{kt}")
        nc.vector.memset(va, 1.0)
        nc.vector.tensor_copy(
            out=va[:, :, 0:DH],
            in_=ps.rearrange("p (h d) -> p h d", h=H))
        v_aug.append(va)

    # ---------------- T matrix (B_D weights) + per-head A_C operands ------
    # per-head qv/qu at partition base 0 (matmul operand base must be 0/32/64)
    qv_h = [setup.tile([DH, Q], FP32, tag=f"qvh{h}", name=f"qvh{h}")
            for h in range(H)]
    qu_hb = [const.tile([DH, Q], BF16, tag=f"quhb{h}", name=f"quhb{h}")
             for h in range(H)]
    for h in range(H):
        dt, r = h // 4, (h % 4) * DH
        nc.vector.tensor_copy(out=qv_h[h], in_=qvT[dt][r:r + DH, :])
        nc.vector.tensor_copy(out=qu_hb[h], in_=quT[dt][r:r + DH, :])

    # T matrices in d-PARITY layout: T_pb[e][c, q, h] = T[2c+e, q, h], to match
    # the u32-packed posT (partition c holds the bf16 pair (d=2c, d=2c+1)).
    T_pb = [const.tile([128, Q, H], BF16, tag=f"T{e}", name=f"Tpb{e}") for e in range(2)]
    for h in range(H):
        Wr_par = Wr_h[h].rearrange("a (c e) -> a c e", e=2)
        for e in range(2):
            ps = psum_big.tile([128, Q], FP32, tag="big", name="ps_projq")
            nc.tensor.matmul(
                ps, Wr_par[:, :, e],
                qv_h[h], start=True, stop=True)
            nc.vector.tensor_copy(out=T_pb[e][:, :, h], in_=ps)

    # ---------------- per-pair pipeline, two phases ----------------
    # Phase A covers pairs [0, 64), phase B [64, 96). Each phase owns three
    # [128, 512] scores tiles (one PSUM bank per j), freeing banks for a
    # 3-deep posT psum ring; B_D for pair p is emitted AFTER pair p+1's
    # transposes so the PSUM->SBUF copy latency never stalls the in-order PE
    # queue.
    exp_sb = [pair_pool.tile([128, H, Q], BF16, tag=f"exp{kt}", name=f"exp{kt}")
              for kt in range(KT)]

    def emit_ac(sc_v, j, p0, p1):
        # A_C: the h==0 matmul opens the psum accumulation group for this
        # (j, phase); the phase's final B_D matmul closes it.
        for h in range(H):
            nc.tensor.matmul(
                sc_v[:, :, h],
                kp_h[h][:, j * 128:(j + 1) * 128],
                qu_hb[h][:, p0:p1],
                start=(h == 0), stop=False)

    def emit_transposes(p, pb):
        """3 u32-packed PE transposes + 1 PSUM->SBUF copy for one pair.

        pb is the pair's bf16 slab [128, KT, D]; viewing consecutive bf16
        d-pairs as one f32 element, each [128, 128-u32] block transpose
        covers all 256 d columns of a k-tile, so 3 transposes cover the pair.
        The fp32 transpose moves the two 16-bit halves exactly.
        """
        pbu = pb.bitcast(FP32)  # [128, KT, 128] packed view
        ps = psum_posT.tile([128, L], FP32, tag="pt", name="pt_ps")
        for j in range(KT):
            nc.tensor.matmul(
                ps[:, j * 128:(j + 1) * 128],
                pbu[:, j, :],
                ident_f,
                is_transpose=True,
                start=(j == 0), stop=(j == KT - 1))
        # copy at bf16 granularity: each 16-bit half is a genuine bf16 value,
        # so no fp32 NaN-pattern canonicalization can corrupt the packed data
        pT = posT_pool.tile([128, L], FP32, tag="posT", name="posT")
        if p % 2 == 0:
            nc.vector.tensor_copy(out=pT.bitcast(BF16), in_=ps.bitcast(BF16))
        else:
            nc.scalar.activation(
                out=pT.bitcast(BF16), in_=ps.bitcast(BF16),
                func=mybir.ActivationFunctionType.Copy)
        # [c, j, k, e] bf16 view: element = posT[d=2c+e, k-tile j, col k]
        return pT.bitcast(BF16).rearrange("c (j k e) -> c j k e", j=KT, e=2)

    def emit_bd(scores, pTv, p, p0, last):
        for e in range(2):
            for j in range(KT):
                nc.tensor.matmul(
                    scores[j][:, (p - p0) * H:(p - p0 + 1) * H],
                    pTv[:, j, :, e],
                    T_pb[e][:, p, :],
                    start=False, stop=(e == 1) and last)

    # pos cast f32 -> bf16 round-robins over Pool/DVE/ACT
    def emit_cast(p, src):
        pb = cast_pool.tile([128, KT, D], BF16, tag="pb", name=f"pb{p % 3}")
        if p % 3 == 0:
            nc.gpsimd.tensor_copy(out=pb, in_=src)
        elif p % 3 == 1:
            nc.vector.tensor_copy(out=pb, in_=src)
        else:
            nc.scalar.activation(
                out=pb, in_=src, func=mybir.ActivationFunctionType.Copy)
        return pb

    PG = 4  # pairs per DMA batch (amortize SP issue cost per dma)
    for p0, p1 in ((0, 64), (64, Q)):
        scores = [scores_pool.tile([128, 512], FP32, tag="sc", name=f"sc{p0}_{j}")
                  for j in range(KT)]
        sc_v = [s[:, :(p1 - p0) * H].rearrange("p (q h) -> p q h", h=H)
                for s in scores]
        for j in range(KT):
            emit_ac(sc_v[j], j, p0, p1)
        pending = None  # transposed pair awaiting its B_D matmuls
        for g in range(p0 // PG, p1 // PG):
            pos_f = pair_pool.tile([128, PG, KT, D], FP32, tag="pos_f")
            # fully contiguous descriptors: partition r reads 3KB runs
            # (k rows {3r, 3r+1, 3r+2} of each pair)
            nc.sync.dma_start(
                out=pos_f,
                in_=pos[g * PG:(g + 1) * PG].rearrange(
                    "g (r j) c -> r g j c", j=KT))
            pbs = [emit_cast(g * PG + i, pos_f[:, i]) for i in range(PG)]
            for i in range(PG):
                pTv = emit_transposes(g * PG + i, pbs[i])
                if pending is not None:
                    emit_bd(scores, pending[0], pending[1], p0, last=False)
                pending = (pTv, g * PG + i)
        emit_bd(scores, pending[0], pending[1], p0, last=True)

        # exp (+scale, +mask) for this phase's pair range
        for j in range(KT):
            nc.scalar.activation(
                out=exp_sb[j].rearrange("p h q -> p q h")[:, p0:p1, :],
                in_=sc_v[j],
                func=mybir.ActivationFunctionType.Exp,
                bias=mbias[j], scale=float(SCALE))

    # ---------------- output matmuls + normalize ----------------
    out_sb = setup.tile([96, D], FP32, tag="osb")
    for h in range(H):
        po = psum_big.tile([DH + 1, Q], FP32, tag="big")
        for kt in range(KT):
            nc.tensor.matmul(
                po, v_aug[kt][:, h, :], exp_sb[kt][:, h, :],
                start=(kt == 0), stop=(kt == KT - 1))
        tmp = pair_pool.tile([DH + 1, Q], FP32, tag="otmp")
        nc.vector.tensor_copy(out=tmp, in_=po)
        pot = psum_big.tile([Q, DH + 1], FP32, tag="big")
        nc.tensor.matmul(
            pot, tmp, ident_f[:DH + 1, :DH + 1],
            is_transpose=True, start=True, stop=True)
        rec = pair_pool.tile([Q, 1], FP32, tag="rec")
        nc.vector.reciprocal(out=rec, in_=pot[:, DH:DH + 1])
        nc.vector.tensor_scalar_mul(
            out=out_sb[:, h * DH:(h + 1) * DH], in0=pot[:, 0:DH], scalar1=rec)

    nc.sync.dma_start(out=out, in_=out_sb)
    ctx.close()


def build_program():
    nc = bacc.Bacc(
        "TRN2", target_bir_lowering=False, debug=False,
        num_devices=NCORES)
    ins = {
        "pos": nc.dram_tensor("pos", [Q, L, D], FP32, kind="ExternalInput").ap(),
        "key": nc.dram_tensor("key", [L, D], FP32, kind="ExternalInput").ap(),
        "value": nc.dram_tensor("value", [L, D], FP32, kind="ExternalInput").ap(),
        "query": nc.dram_tensor("query", [Q, D], FP32, kind="ExternalInput").ap(),
        "mask": nc.dram_tensor("mask", [L], FP32, kind="ExternalInput").ap(),
        "Wk": nc.dram_tensor("Wk", [D, D], FP32, kind="ExternalInput").ap(),
        "Wq": nc.dram_tensor("Wq", [D, D], FP32, kind="ExternalInput").ap(),
        "Wv": nc.dram_tensor("Wv", [D, D], FP32, kind="ExternalInput").ap(),
        "Wr": nc.dram_tensor("Wr", [D, D], FP32, kind="ExternalInput").ap(),
        "bk": nc.dram_tensor("bk", [D], FP32, kind="ExternalInput").ap(),
        "bq": nc.dram_tensor("bq", [D], FP32, kind="ExternalInput").ap(),
        "bv": nc.dram_tensor("bv", [D], FP32, kind="ExternalInput").ap(),
        "u": nc.dram_tensor("u", [H, DH], FP32, kind="ExternalInput").ap(),
        "v": nc.dram_tensor("v", [H, DH], FP32, kind="ExternalInput").ap(),
    }
    outs = {
        "out": nc.dram_tensor("out", [Q, D], FP32, kind="ExternalOutput").ap(),
    }
    with tile.TileContext(nc) as tc:
        build_kernel_body(tc, outs, ins)
    nc.compile()
    return nc


def shard_inputs(inputs):
    """Full inputs -> list of 8 per-core input dicts (numpy, contiguous)."""
    f32 = lambda a: np.ascontiguousarray(np.asarray(a), dtype=np.float32)
    pos = f32(inputs["pos"])
    key = f32(inputs["key"])
    query = f32(inputs["query"])
    value = f32(inputs["value"])
    mask = f32(inputs["key_mask"])
    shared = {
        "Wk": f32(inputs["Wk"]), "Wq": f32(inputs["Wq"]),
        "Wv": f32(inputs["Wv"]), "Wr": f32(inputs["Wr"]),
        "bk": f32(inputs["bk"]), "bq": f32(inputs["bq"]),
        "bv": f32(inputs["bv"]),
        "u": f32(inputs["u"]), "v": f32(inputs["v"]),
    }
    in_maps = []
    for c in range(NCORES):
        b, q0 = c // 4, (c % 4) * Q
        m = dict(shared)
        m["pos"] = np.ascontiguousarray(pos[b, q0:q0 + Q])
        m["key"] = key[b]
        m["value"] = value[b]
        m["query"] = np.ascontiguousarray(query[b, q0:q0 + Q])
        m["mask"] = mask[b]
        in_maps.append(m)
    return in_maps


_CACHED = {}


def kernel(**inputs):
    from concourse.bass_utils import run_bass_kernel_spmd

    if "nc" not in _CACHED:
        _CACHED["nc"] = build_program()
    nc = _CACHED["nc"]
    in_maps = shard_inputs(inputs)
    res = run_bass_kernel_spmd(nc, in_maps, core_ids=list(range(NCORES)))
    out = np.zeros((B, L, D), dtype=np.float32)
    for c in range(NCORES):
        b, q0 = c // 4, (c % 4) * Q
        out[b, q0:q0 + Q] = res.results[c]["out"]
    return out

